# revision 12
# baseline (speedup 1.0000x reference)
"""Causal no-head self-attention with RoPE on 8 Trainium2 NeuronCores.

Sharding: 8 cores = 4 batches x 2 query-sets of four 256-query blocks.
Per-core slots s=0..3 run 4(s+1) key-tile visits (128 keys each); block
assignment (h=0: blocks {1,3,4,7}, h=1: {0,2,5,6}) makes the same
(4,8,12,16) visit structure causally sufficient on every core, so all
cores run ONE identical Bass program; per-core differences (which
queries, which keys, causal masks, RoPE angles) are carried in the
input data.

K/V projection is deduplicated across the two cores sharing a batch:
core h projects (and RoPEs) keys [512h,512h+512) u [1024+512h,...+512)
only, and the pair exchanges 512-key chunks with four small in-pair
AllGathers (K0, V0, K1, V1) that pipeline against the V/Q projections.
All HBM traffic uses few large fused DMA descriptors (a single
descriptor spreads over all 16 SDMA engines) to keep trigger overhead
off the engine streams.  Everything is bf16 (PE full rate) with fp32
PSUM accumulation; K^T/V/Q^T live entirely in SBUF.  QT/KT are
[d_k, cols] with d_k de-interleaved (even feats rows 0:512, odd
512:1024) so RoPE is a contiguous-partition-block rotation; the same
permutation is applied to Wq/Wk output columns on host.

Softmax normalization is deferred: attnT holds unnormalized PV sums and
the 1/rowsum factor (PE-broadcast across partitions, pipelined in two
halves across slots) is applied by the output-projection PSUM eviction.
Output projection quarters are interleaved into the slot loop so the PE
stream has no bubbles at slot boundaries.
"""

import numpy as np
import sys

for _p in ("/opt/trn_rl_repo",):
    if _p not in sys.path:
        sys.path.insert(0, _p)

import concourse.bass as bass
import concourse.bacc as bacc
import concourse.mybir as mybir
from concourse.tile import TileContext
from concourse.bass_utils import run_bass_kernel_spmd

B, S, D = 4, 2048, 1024
THETA = 10000.0
P = 128
NT = D // P          # 8 partition-tiles over the 1024 dim
SB = 512             # seq block width for K/V projection
QB = 256             # query slot width (4 slots per core)
NQ = 1024            # queries (and local keys) per core
F32 = mybir.dt.float32
F32R = mybir.dt.float32r
BF16 = mybir.dt.bfloat16
SCALE = 1.0 / 32.0   # 1/sqrt(D)
BLOCKS = [[1, 3, 4, 7], [0, 2, 5, 6]]   # 256-query blocks per core half
NVIS = [4, 8, 12, 16]                    # kt visits per slot (same all cores)


def _build_program():
    nc = bacc.Bacc("TRN2", num_swdge_queues=4)
    inp = {}
    def din(name, shape, dt):
        inp[name] = nc.dram_tensor(name, shape, dt, kind="ExternalInput")
    # all inputs are host-repacked chunk/partition-major ([ch, P, rows, 512])
    # so every load descriptor reads long contiguous DRAM runs per partition
    din("xTm", [2, P, NT, SB], BF16)
    din("xTq", [2, P, NT, SB], BF16)
    din("WqT", [2, P, NT, SB], BF16)
    din("WkT", [2, P, NT, SB], BF16)
    din("WvT", [2, P, NT, SB], BF16)
    din("WoT", [2, P, NT, SB], BF16)
    din("cosM", [2, P, NT // 2, SB], BF16)
    din("sinM", [2, P, NT // 2, SB], BF16)
    din("cosQ", [2, P, NT // 2, SB], BF16)
    din("sinQ", [2, P, NT // 2, SB], BF16)
    din("masks", [P, 16, QB], BF16)
    din("ones_col", [P, 1], F32R)
    din("ones_row", [1, P], F32R)
    outT = nc.dram_tensor("outT", [P, NT, NQ], BF16, kind="ExternalOutput")

    from contextlib import ExitStack
    with TileContext(nc) as tc:
        with ExitStack() as ctx:
            pool = lambda *a, **kw: ctx.enter_context(tc.tile_pool(*a, **kw))
            res = pool(name="res", bufs=1)          # big residents
            dpool = pool(name="dram", bufs=1, space="DRAM")
            wres = pool(name="wres", bufs=1)        # weights (wk->wq, wv->wo)
            smp = pool(name="small", bufs=1)
            xbp = pool(name="xb", bufs=3)
            csp = pool(name="cs", bufs=2)
            rawp = pool(name="raw", bufs=2)
            tmpp = pool(name="tmp", bufs=1)
            ptp = pool(name="pt", bufs=1)
            sap = pool(name="sa", bufs=1)
            bcp = pool(name="bc", bufs=2)
            obp = pool(name="ob", bufs=2)
            psB = pool(name="psB", bufs=3, space="PSUM")
            psPV = pool(name="psPV", bufs=4, space="PSUM")
            sps = pool(name="sums", bufs=1, space="PSUM")

            KT = res.tile([P, NT, S], BF16, tag="kt")
            V = res.tile([P, S // P, D], BF16, tag="v")
            # QT doubles as attnT: slot s's unnormalized PV overwrites QT's
            # columns after the slot's scores are done reading them.
            QT = res.tile([P, NT, NQ], BF16, tag="qt")
            attnT = QT
            maskst = res.tile([P, 16, QB], BF16, tag="msk")
            ones_col = smp.tile([P, 1], F32R, tag="onescol")
            ones_row = smp.tile([1, P], F32R, tag="onesrow")

            # ---------- HAM warmup: junk matmuls so the PE clock is at 2.4
            # GHz (K=8/8) by the time real data lands ----------------------
            junk = smp.tile([P, SB], BF16, tag="junk")
            nc.gpsimd.memset(junk[:], 0.0)
            wps = psB.tile([P, SB], F32, tag="psB")
            NWARM = 10
            for i in range(NWARM):
                nc.tensor.matmul(wps[:], junk[:, 0:P], junk[:],
                                 start=(i == 0), stop=(i == NWARM - 1))
            nc.scalar.copy(junk[:], wps[:])

            # ---------- startup loads --------------------------------------
            # Packets round-robin across the two HWDGE rings, so ONLY the
            # first matmul group's data (xm0 on sync, Wk halves on scalar)
            # is enqueued first; everything else follows.
            xms = [xbp.tile([P, NT, SB], BF16, tag="xb", name=f"xm{j}")
                   for j in range(2)]
            cms = [csp.tile([P, NT // 2, SB], BF16, tag="cs_c", name=f"cm{j}")
                   for j in range(2)]
            sms = [csp.tile([P, NT // 2, SB], BF16, tag="cs_s", name=f"sm{j}")
                   for j in range(2)]
            nc.sync.dma_start(xms[0][:], inp["xTm"][0])
            WkRes = wres.tile([P, NT, D], BF16, tag="wk")
            WvRes = wres.tile([P, NT, D], BF16, tag="wv")
            for hh in range(2):
                w_sl = slice(hh * SB, (hh + 1) * SB)
                nc.scalar.dma_start(WkRes[:, :, w_sl], inp["WkT"][hh])
            nc.sync.dma_start(cms[0][:], inp["cosM"][0])
            nc.sync.dma_start(sms[0][:], inp["sinM"][0])
            nc.sync.dma_start(xms[1][:], inp["xTm"][1])
            nc.sync.dma_start(ones_col[:], inp["ones_col"][:])
            nc.sync.dma_start(ones_row[:], inp["ones_row"][:])
            xq0 = xbp.tile([P, NT, SB], BF16, tag="xb")
            nc.sync.dma_start(xq0[:], inp["xTq"][0])
            nc.sync.dma_start(cms[1][:], inp["cosM"][1])
            nc.sync.dma_start(sms[1][:], inp["sinM"][1])
            cq0 = csp.tile([P, NT // 2, SB], BF16, tag="cs_c")
            sq0 = csp.tile([P, NT // 2, SB], BF16, tag="cs_s")
            nc.sync.dma_start(cq0[:], inp["cosQ"][0])
            nc.sync.dma_start(sq0[:], inp["sinQ"][0])
            nc.sync.dma_start(maskst[:], inp["masks"][:])
            # late Q-phase loads: gated only by pool-buffer reuse, so they
            # stay on the sync ring ahead of the outT stores
            cq1 = csp.tile([P, NT // 2, SB], BF16, tag="cs_c")
            sq1 = csp.tile([P, NT // 2, SB], BF16, tag="cs_s")
            nc.sync.dma_start(cq1[:], inp["cosQ"][1])
            nc.sync.dma_start(sq1[:], inp["sinQ"][1])
            xq1 = xbp.tile([P, NT, SB], BF16, tag="xb")
            nc.sync.dma_start(xq1[:], inp["xTq"][1])
            for hh in range(2):
                w_sl = slice(hh * SB, (hh + 1) * SB)
                nc.scalar.dma_start(WvRes[:, :, w_sl], inp["WvT"][hh])

            def rope_block(dst, src, cos_t, sin_t):
                # dst/src: [P, NT, w]; rows 0:NT/2 = even feats, NT/2: = odd
                h = NT // 2
                w = src.shape[-1]
                e, o = src[:, 0:h, :], src[:, h:NT, :]
                c, s = cos_t[:, :, :], sin_t[:, :, :]
                t1 = tmpp.tile([P, h, SB], BF16, tag="t1")
                nc.vector.tensor_mul(out=dst[:, 0:h, :], in0=e, in1=c)
                nc.vector.tensor_mul(out=t1[:, :, 0:w], in0=o, in1=s)
                nc.vector.tensor_tensor(dst[:, 0:h, :], dst[:, 0:h, :],
                                        t1[:, :, 0:w], mybir.AluOpType.subtract)
                t2 = tmpp.tile([P, h, SB], BF16, tag="t1")
                nc.vector.tensor_mul(out=dst[:, h:NT, :], in0=o, in1=c)
                nc.vector.tensor_mul(out=t2[:, :, 0:w], in0=e, in1=s)
                nc.vector.tensor_tensor(dst[:, h:NT, :], dst[:, h:NT, :],
                                        t2[:, :, 0:w], mybir.AluOpType.add)

            # ---------- Phase 0: K/V projection of MY half of the keys ------
            # jb-th local chunk = global seq block 2*jb + rank; one 2MB
            # in-pair AllGather per chunk (K rows 0:8, V rows 8:16).
            RG = [[0, 1], [2, 3], [4, 5], [6, 7]]
            kvin = [dpool.tile([P, 16, SB], BF16, name=f"kvin{j}") for j in range(2)]
            kvout = [dpool.tile([2, P, 16, SB], BF16, name=f"kvout{j}") for j in range(2)]

            for jb in range(2):
                sl = slice(jb * SB, (jb + 1) * SB)
                xb = xms[jb]
                # K projection of my 512-key chunk
                kraw = rawp.tile([P, NT, SB], BF16, tag="raw")
                for t_out in range(NT):
                    ps = psB.tile([P, SB], F32, tag="psB")
                    for dt_ in range(NT):
                        nc.tensor.matmul(ps[:], WkRes[:, dt_, t_out * P:(t_out + 1) * P],
                                         xb[:, dt_, :], start=(dt_ == 0), stop=(dt_ == NT - 1))
                    nc.vector.tensor_copy(kraw[:, t_out, :], ps[:])
                rope_block(KT[:, :, sl], kraw, cms[jb], sms[jb])
                nc.scalar.dma_start(kvin[jb][:, 0:NT, :], KT[:, :, sl])
                if jb == 1:
                    # WkRes dead after this chunk's K matmuls: reuse for Wq
                    WqRes = wres.tile([P, NT, D], BF16, tag="wk")
                    for hh in range(2):
                        w_sl = slice(hh * SB, (hh + 1) * SB)
                        nc.scalar.dma_start(WqRes[:, :, w_sl], inp["WqT"][hh])
                # V projection of my 512-key chunk
                for sk_ in range(SB // P):
                    for dh in range(2):
                        ps = psB.tile([P, SB], F32, tag="psB")
                        for dt_ in range(NT):
                            nc.tensor.matmul(ps[:], xb[:, dt_, sk_ * P:(sk_ + 1) * P],
                                             WvRes[:, dt_, dh * SB:(dh + 1) * SB],
                                             start=(dt_ == 0), stop=(dt_ == NT - 1))
                        nc.scalar.copy(V[:, jb * (SB // P) + sk_, dh * SB:(dh + 1) * SB], ps[:])
                nc.scalar.dma_start(kvin[jb][:, NT:16, :],
                                    V[:, jb * (SB // P):(jb + 1) * (SB // P), :])
                nc.gpsimd.collective_compute(
                    "AllGather", mybir.AluOpType.bypass, replica_groups=RG,
                    ins=[kvin[jb].opt()], outs=[kvout[jb].opt()])
                if jb == 1:
                    # WvRes dead: reuse for Wo
                    WoRes = wres.tile([P, NT, D], BF16, tag="wv")
                    for hh in range(2):
                        w_sl = slice(hh * SB, (hh + 1) * SB)
                        nc.scalar.dma_start(WoRes[:, :, w_sl], inp["WoT"][hh])

            # ---------- gather AG results back to SBUF (gpsimd queues, so
            # the AG-gated waits never block the sync/scalar rings) ---------
            for jb in range(2):
                for r in range(2):
                    g = 2 * jb + r
                    nc.gpsimd.dma_start(KT[:, :, g * SB:(g + 1) * SB],
                                        kvout[jb][r, :, 0:NT, :])
                    nc.gpsimd.dma_start(V[:, g * (SB // P):(g + 1) * (SB // P), :],
                                        kvout[jb][r, :, NT:16, :])

            # ---------- Phase 1: Q^T projection + RoPE (overlaps the AGs) ---
            for qh, (xq, cq, sq) in enumerate(((xq0, cq0, sq0), (xq1, cq1, sq1))):
                sl = slice(qh * SB, (qh + 1) * SB)
                qraw = rawp.tile([P, NT, SB], BF16, tag="raw")
                for t_out in range(NT):
                    ps = psB.tile([P, SB], F32, tag="psB")
                    for dt_ in range(NT):
                        nc.tensor.matmul(ps[:], WqRes[:, dt_, t_out * P:(t_out + 1) * P],
                                         xq[:, dt_, :], start=(dt_ == 0), stop=(dt_ == NT - 1))
                    nc.vector.tensor_copy(qraw[:, t_out, :], ps[:])
                rope_block(QT[:, :, sl], qraw, cq, sq)

            # ---------- attention + output projection, software-pipelined ---
            sumrows = {}
            rcps = {}

            def bc_part1(s, sumacc):
                # rowsum of slot s across partitions -> [1, QB] (PE trick)
                sums_ps = sps.tile([1, SB], F32, tag="sums")
                nc.tensor.matmul(sums_ps[0:1, 0:QB], ones_col[:], sumacc[:],
                                 start=True, stop=True)
                sumrow = smp.tile([1, QB], F32R, tag="sumrow")
                nc.scalar.copy(sumrow[:], sums_ps[0:1, 0:QB])
                sumrows[s] = sumrow

            def bc_part2(s):
                # broadcast rowsum to all partitions, reciprocal -> rcps[s]
                bc_ps = psB.tile([P, SB], F32, tag="psB")
                nc.tensor.matmul(bc_ps[:, 0:QB], ones_row[:], sumrows[s][:],
                                 start=True, stop=True)
                bc_sums = bcp.tile([P, QB], F32R, tag="bcs")
                nc.scalar.copy(bc_sums[:], bc_ps[:, 0:QB])
                rcp = bcp.tile([P, QB], F32, tag="bc")
                nc.vector.reciprocal(rcp[:], bc_sums[:])
                rcps[s] = rcp

            def waveB(sp, pc, pqsl, pPT):
                # PV for d-tiles 4..7 of the previous slot
                pvB = [psPV.tile([P, SB], F32, tag="pv", name=f"pvB{sp}_{j}")
                       for j in range(4)]
                for v in range(pc):
                    for j in range(4):
                        nc.tensor.matmul(pvB[j][:, 0:QB],
                                         V[:, v, (4 + j) * P:(5 + j) * P],
                                         pPT[:, v, :], start=(v == 0),
                                         stop=(v == pc - 1))
                for j in range(4):
                    nc.scalar.copy(attnT[:, 4 + j, pqsl], pvB[j][:, 0:QB])

            def o_quarter(qh):
                # output projection of slot qh's queries; normalization
                # (1/rowsum) fused into the PSUM eviction
                sl = slice(qh * QB, (qh + 1) * QB)
                ob = obp.tile([P, NT, QB], BF16, tag="ob")
                for oc in range(NT):
                    ps = psB.tile([P, SB], F32, tag="psB")
                    for dt_ in range(NT):
                        nc.tensor.matmul(ps[:, 0:QB], WoRes[:, dt_, oc * P:(oc + 1) * P],
                                         attnT[:, dt_, sl], start=(dt_ == 0), stop=(dt_ == NT - 1))
                    nc.vector.tensor_mul(out=ob[:, oc, :], in0=ps[:, 0:QB], in1=rcps[qh][:])
                    nc.sync.dma_start(outT[:, oc, sl], ob[:, oc, :])

            prev = None
            for s in range(4):
                if prev is not None:
                    waveB(s - 1, *prev)
                c = NVIS[s]
                qsl = slice(s * QB, (s + 1) * QB)
                sumacc = sap.tile([P, QB], F32R, tag="sa")
                PT = ptp.tile([P, 16, QB], BF16, tag="pts")
                pvA = [psPV.tile([P, SB], F32, tag="pv", name=f"pvA{s}_{j2}")
                       for j2 in range(4)]
                for v in range(c):
                    ps = psB.tile([P, SB], F32, tag="psB")
                    for dt_ in range(NT):
                        nc.tensor.matmul(ps[:, 0:QB], KT[:, dt_, v * P:(v + 1) * P],
                                         QT[:, dt_, qsl], start=(dt_ == 0), stop=(dt_ == NT - 1))
                    nc.scalar.activation(PT[:, v, :], ps[:, 0:QB],
                                         mybir.ActivationFunctionType.Exp, scale=SCALE)
                    if v >= c - 4:
                        nc.vector.tensor_mul(out=PT[:, v, :], in0=PT[:, v, :],
                                             in1=maskst[:, 4 * s + (v - (c - 4)), :])
                    if v == 0:
                        nc.vector.tensor_copy(sumacc[:], PT[:, v, :])
                    else:
                        nc.vector.tensor_tensor(sumacc[:], sumacc[:], PT[:, v, :],
                                                mybir.AluOpType.add)
                    if v > 0:
                        for j2 in range(4):
                            nc.tensor.matmul(pvA[j2][:, 0:QB],
                                             V[:, v - 1, j2 * P:(j2 + 1) * P], PT[:, v - 1, :],
                                             start=(v - 1 == 0), stop=False)
                for j2 in range(4):
                    nc.tensor.matmul(pvA[j2][:, 0:QB],
                                     V[:, c - 1, j2 * P:(j2 + 1) * P], PT[:, c - 1, :],
                                     start=(c == 1), stop=True)
                bc_part1(s, sumacc)
                for j2 in range(4):
                    nc.vector.tensor_copy(attnT[:, j2, qsl], pvA[j2][:, 0:QB])
                if s >= 1:
                    bc_part2(s - 1)
                    o_quarter(s - 1)
                prev = (c, qsl, PT)

            # ---------- wave B of the last slot + final output quarter ------
            waveB(3, *prev)
            bc_part2(3)
            o_quarter(3)

    nc.finalize()
    return nc


def _chunk_pack(a, w=SB):
    # a: [R*P, C] -> [C//w, P, R, w] so each (chunk, partition) row is one
    # contiguous DRAM run (R*w elements) for line-rate DMA descriptors
    R = a.shape[0] // P
    nch = a.shape[1] // w
    return np.ascontiguousarray(a.reshape(R, P, nch, w).transpose(2, 1, 0, 3))


def _host_inputs(x, Wq, Wk, Wv, Wo, token_positions):
    import ml_dtypes
    bf = ml_dtypes.bfloat16
    perm = np.concatenate([np.arange(0, D, 2), np.arange(1, D, 2)])
    WqTp = _chunk_pack(Wq[perm].T.astype(bf))
    WkTp = _chunk_pack(Wk[perm].T.astype(bf))
    WvT = _chunk_pack(Wv.T.astype(bf))
    WoT = _chunk_pack(Wo.T.astype(bf))
    inv_freq = (1.0 / (np.float32(THETA) **
                       (np.arange(0, D, 2, dtype=np.float32) / np.float32(D))))
    ones_col = np.ones((P, 1), np.float32)
    ones_row = np.ones((1, P), np.float32)

    in_maps, metas = [], []
    for b in range(B):
        xT = np.ascontiguousarray(x[b].T).astype(bf)           # [D, S]
        pos = token_positions[b].astype(np.float32)
        ang = (pos[None, :] * inv_freq[:, None]).astype(np.float32)  # [D/2, S]
        cosF = np.cos(ang)
        sinF = np.sin(ang)
        for h in range(2):
            blocks = BLOCKS[h]
            qcols = np.concatenate([np.arange(QB * bs, QB * (bs + 1))
                                    for bs in blocks])
            xTq = _chunk_pack(xT[:, qcols])
            cosQ = _chunk_pack(cosF[:, qcols].astype(bf))
            sinQ = _chunk_pack(sinF[:, qcols].astype(bf))
            # my key half: global seq blocks h and 2+h (512 keys each)
            mcols = np.concatenate([np.arange(SB * h, SB * (h + 1)),
                                    np.arange(1024 + SB * h, 1024 + SB * (h + 1))])
            xTm = _chunk_pack(xT[:, mcols])
            cosM = _chunk_pack(cosF[:, mcols].astype(bf))
            sinM = _chunk_pack(sinF[:, mcols].astype(bf))
            m = np.zeros((P, 16, QB), dtype=np.float32)
            for s, bs in enumerate(blocks):
                c = NVIS[s]
                q0 = QB * bs
                q_glob = q0 + np.arange(QB)
                for j in range(4):
                    v = c - 4 + j
                    k_glob = 128 * v + np.arange(P)
                    m[:, 4 * s + j, :] = (q_glob[None, :] >= k_glob[:, None])
            in_maps.append({
                "ones_col": ones_col, "ones_row": ones_row,
                "xTm": xTm, "xTq": xTq,
                "WqT": WqTp, "WkT": WkTp, "WvT": WvT, "WoT": WoT,
                "cosM": cosM, "sinM": sinM,
                "cosQ": cosQ, "sinQ": sinQ,
                "masks": m.astype(bf),
            })
            metas.append((b, qcols))
    return in_maps, metas


_NC_CACHE = {}


def kernel(x, Wq, Wk, Wv, Wo, token_positions):
    x = np.asarray(x); token_positions = np.asarray(token_positions)
    if "nc" not in _NC_CACHE:
        _NC_CACHE["nc"] = _build_program()
    nc = _NC_CACHE["nc"]
    in_maps, metas = _host_inputs(np.asarray(x), np.asarray(Wq), np.asarray(Wk),
                                  np.asarray(Wv), np.asarray(Wo), token_positions)
    res = run_bass_kernel_spmd(nc, in_maps, core_ids=list(range(8)))
    out = np.empty((B, S, D), dtype=np.float32)
    for (b, qcols), r in zip(metas, res.results):
        oT = np.asarray(r["outT"]).astype(np.float32)   # [P, NT, NQ]
        o = np.transpose(oT, (2, 1, 0)).reshape(NQ, D)
        out[b, qcols, :] = o
    return out


# revision 35
# speedup vs baseline: 1.1185x; 1.1185x over previous
"""Causal no-head self-attention with RoPE on 8 Trainium2 NeuronCores.

Sharding: 8 cores = 4 batches x 2 query-sets of four 256-query blocks.
Per-core slots s=0..3 run 4(s+1) key-tile visits (128 keys each); block
assignment (h=0: blocks {1,3,4,7}, h=1: {0,2,5,6}) makes the same
(4,8,12,16) visit structure causally sufficient on every core, so all
cores run ONE identical Bass program; per-core differences (which
queries, which keys, causal masks, RoPE angles) are carried in the
input data.

K/V projection is deduplicated across the two cores sharing a batch:
core h projects (and RoPEs) keys [512h,512h+512) u [1024+512h,...+512)
only, and the pair exchanges 512-key chunks with four small in-pair
AllGathers (K0, V0, K1, V1) that pipeline against the V/Q projections.
All HBM traffic uses few large fused DMA descriptors (a single
descriptor spreads over all 16 SDMA engines) to keep trigger overhead
off the engine streams.  Everything is bf16 (PE full rate) with fp32
PSUM accumulation; K^T/V/Q^T live entirely in SBUF.  QT/KT are
[d_k, cols] with d_k de-interleaved (even feats rows 0:512, odd
512:1024) so RoPE is a contiguous-partition-block rotation; the same
permutation is applied to Wq/Wk output columns on host.

Softmax normalization is deferred: attnT holds unnormalized PV sums and
the 1/rowsum factor (PE-broadcast across partitions, pipelined in two
halves across slots) is applied by the output-projection PSUM eviction.
Output projection quarters are interleaved into the slot loop so the PE
stream has no bubbles at slot boundaries.
"""

import numpy as np
import sys

for _p in ("/opt/trn_rl_repo",):
    if _p not in sys.path:
        sys.path.insert(0, _p)

import concourse.bass as bass
import concourse.bacc as bacc
import concourse.mybir as mybir
from concourse.tile import TileContext
from concourse.bass_utils import run_bass_kernel_spmd

B, S, D = 4, 2048, 1024
THETA = 10000.0
P = 128
NT = D // P          # 8 partition-tiles over the 1024 dim
SB = 512             # seq block width for K/V projection
QB = 256             # query slot width (4 slots per core)
NQ = 1024            # queries (and local keys) per core
F32 = mybir.dt.float32
F32R = mybir.dt.float32r
BF16 = mybir.dt.bfloat16
SCALE = 1.0 / 32.0   # 1/sqrt(D)
BLOCKS = [[1, 3, 4, 7], [0, 2, 5, 6]]   # 256-query blocks per core half
NVIS = [4, 8, 12, 16]                    # kt visits per slot (same all cores)


def _build_program():
    nc = bacc.Bacc("TRN2", num_swdge_queues=4)
    inp = {}
    def din(name, shape, dt):
        inp[name] = nc.dram_tensor(name, shape, dt, kind="ExternalInput")
    # all inputs are host-repacked chunk/partition-major ([ch, P, rows, 512])
    # so every load descriptor reads long contiguous DRAM runs per partition
    din("xTm", [2, P, NT, SB], BF16)
    din("xTq", [2, P, NT, SB], BF16)
    din("WqT", [2, P, NT, SB], BF16)
    din("WkT", [4, P, NT, SB // 2], BF16)
    din("WvT", [2, P, NT, SB], BF16)
    din("WoT", [2, P, NT, SB], BF16)
    din("cosM", [2, P, NT // 2, SB], BF16)
    din("sinM", [2, P, NT // 2, SB], BF16)
    din("cosQ", [2, P, NT // 2, SB], BF16)
    din("sinQ", [2, P, NT // 2, SB], BF16)
    din("masks", [P, 16, QB], BF16)
    din("ones_col", [P, 1], F32R)
    din("ones_row", [1, P], F32R)
    outT = nc.dram_tensor("outT", [P, NT, NQ], BF16, kind="ExternalOutput")

    from contextlib import ExitStack
    with TileContext(nc) as tc:
        with ExitStack() as ctx:
            pool = lambda *a, **kw: ctx.enter_context(tc.tile_pool(*a, **kw))
            res = pool(name="res", bufs=1)          # big residents
            dpool = pool(name="dram", bufs=1, space="DRAM")
            wres = pool(name="wres", bufs=1)        # weights (wk->wq, wv->wo)
            smp = pool(name="small", bufs=1)
            xbp = pool(name="xb", bufs=3)
            csp = pool(name="cs", bufs=2)
            rawp = pool(name="raw", bufs=1)
            tmpp = pool(name="tmp", bufs=1)
            ptp = pool(name="pt", bufs=4)
            sap = pool(name="sa", bufs=2)
            bcp = pool(name="bc", bufs=2)
            obp = pool(name="ob", bufs=2)
            psB = pool(name="psB", bufs=3, space="PSUM")
            psPV = pool(name="psPV", bufs=4, space="PSUM")
            sps = pool(name="sums", bufs=1, space="PSUM")

            KT = res.tile([P, NT, S], BF16, tag="kt")
            V = res.tile([P, S // P, D], BF16, tag="v")
            # QT doubles as attnT: slot s's unnormalized PV overwrites QT's
            # columns after the slot's scores are done reading them.
            QT = res.tile([P, NT, NQ], BF16, tag="qt")
            attnT = QT
            maskst = res.tile([P, 16, QB], BF16, tag="msk")
            ones_col = smp.tile([P, 1], F32R, tag="onescol")
            ones_row = smp.tile([1, P], F32R, tag="onesrow")

            # ---------- HAM warmup: junk matmuls so the PE clock is at 2.4
            # GHz (K=8/8) by the time real data lands ----------------------
            junk = smp.tile([P, SB], BF16, tag="junk")
            nc.gpsimd.memset(junk[:], 0.0)
            wps = psB.tile([P, SB], F32, tag="psB")
            NWARM = 20
            for i in range(NWARM):
                nc.tensor.matmul(wps[:], junk[:, 0:P], junk[:],
                                 start=(i == 0), stop=(i == NWARM - 1))
            nc.scalar.copy(junk[:], wps[:])
            # dummy collective to absorb the CC-core wakeup latency before
            # the first real AllGather
            dumin = dpool.tile([P, 1], F32R, name="dumin")
            dumout = dpool.tile([2, P, 1], F32R, name="dumout")
            nc.gpsimd.collective_compute(
                "AllGather", mybir.AluOpType.bypass,
                replica_groups=[[0, 1], [2, 3], [4, 5], [6, 7]],
                ins=[dumin.opt()], outs=[dumout.opt()])

            # ---------- startup loads --------------------------------------
            # Packets round-robin across the two HWDGE rings, so ONLY the
            # first matmul group's data (xm0 on sync, Wk halves on scalar)
            # is enqueued first; everything else follows.
            xms = [xbp.tile([P, NT, SB], BF16, tag="xb", name=f"xm{j}")
                   for j in range(2)]
            cms = [csp.tile([P, NT // 2, SB], BF16, tag="cs_c", name=f"cm{j}")
                   for j in range(2)]
            sms = [csp.tile([P, NT // 2, SB], BF16, tag="cs_s", name=f"sm{j}")
                   for j in range(2)]
            nc.sync.dma_start(xms[0][:], inp["xTm"][0])
            WkRes = wres.tile([P, NT, D], BF16, tag="wk")
            WvRes = wres.tile([P, NT, D], BF16, tag="wv")
            for hh in range(4):
                w_sl = slice(hh * (SB // 2), (hh + 1) * (SB // 2))
                nc.scalar.dma_start(WkRes[:, :, w_sl], inp["WkT"][hh])
            nc.sync.dma_start(cms[0][:], inp["cosM"][0])
            nc.sync.dma_start(sms[0][:], inp["sinM"][0])
            nc.sync.dma_start(xms[1][:], inp["xTm"][1])
            nc.sync.dma_start(ones_col[:], inp["ones_col"][:])
            nc.sync.dma_start(ones_row[:], inp["ones_row"][:])
            xq0 = xbp.tile([P, NT, SB], BF16, tag="xb")
            nc.sync.dma_start(xq0[:], inp["xTq"][0])
            nc.sync.dma_start(cms[1][:], inp["cosM"][1])
            nc.sync.dma_start(sms[1][:], inp["sinM"][1])
            cq0 = csp.tile([P, NT // 2, SB], BF16, tag="cs_c")
            sq0 = csp.tile([P, NT // 2, SB], BF16, tag="cs_s")
            nc.sync.dma_start(cq0[:], inp["cosQ"][0])
            nc.sync.dma_start(sq0[:], inp["sinQ"][0])
            nc.sync.dma_start(maskst[:], inp["masks"][:])
            # late Q-phase loads: gated only by pool-buffer reuse, so they
            # stay on the sync ring ahead of the outT stores
            cq1 = csp.tile([P, NT // 2, SB], BF16, tag="cs_c")
            sq1 = csp.tile([P, NT // 2, SB], BF16, tag="cs_s")
            nc.sync.dma_start(cq1[:], inp["cosQ"][1])
            nc.sync.dma_start(sq1[:], inp["sinQ"][1])
            xq1 = xbp.tile([P, NT, SB], BF16, tag="xb")
            nc.sync.dma_start(xq1[:], inp["xTq"][1])
            for hh in range(2):
                w_sl = slice(hh * SB, (hh + 1) * SB)
                nc.scalar.dma_start(WvRes[:, :, w_sl], inp["WvT"][hh])

            def rope_block(dst, src, cos_t, sin_t):
                # dst/src: [P, NT, w]; rows 0:NT/2 = even feats, NT/2: = odd
                h = NT // 2
                w = src.shape[-1]
                e, o = src[:, 0:h, :], src[:, h:NT, :]
                c, s = cos_t[:, :, :], sin_t[:, :, :]
                t1 = tmpp.tile([P, h, SB], BF16, tag="t1")
                nc.vector.tensor_mul(out=dst[:, 0:h, :], in0=e, in1=c)
                nc.vector.tensor_mul(out=t1[:, :, 0:w], in0=o, in1=s)
                nc.vector.tensor_tensor(dst[:, 0:h, :], dst[:, 0:h, :],
                                        t1[:, :, 0:w], mybir.AluOpType.subtract)
                t2 = tmpp.tile([P, h, SB], BF16, tag="t1")
                nc.vector.tensor_mul(out=dst[:, h:NT, :], in0=o, in1=c)
                nc.vector.tensor_mul(out=t2[:, :, 0:w], in0=e, in1=s)
                nc.vector.tensor_tensor(dst[:, h:NT, :], dst[:, h:NT, :],
                                        t2[:, :, 0:w], mybir.AluOpType.add)

            # ---------- Phase 0: K/V projection of MY half of the keys ------
            # jb-th local chunk = global seq block 2*jb + rank; four 1MB
            # in-pair AllGathers (K0, V0, K1, V1), each triggered the moment
            # its chunk is spilled, pipeline on the CC cores while the PE
            # continues projecting.
            RG = [[0, 1], [2, 3], [4, 5], [6, 7]]
            kvinK = [dpool.tile([P, NT, SB], BF16, name=f"kvinK{j}") for j in range(2)]
            kvinV = [dpool.tile([P, NT, SB], BF16, name=f"kvinV{j}") for j in range(2)]
            kvoutK = [dpool.tile([2, P, NT, SB], BF16, name=f"kvoutK{j}") for j in range(2)]
            kvoutV = [dpool.tile([2, P, NT, SB], BF16, name=f"kvoutV{j}") for j in range(2)]

            for jb in range(2):
                sl = slice(jb * SB, (jb + 1) * SB)
                xb = xms[jb]
                # K projection of my 512-key chunk
                kraw = rawp.tile([P, NT, SB], BF16, tag="raw")
                for t_out in range(NT):
                    ps = psB.tile([P, SB], F32, tag="psB")
                    for dt_ in range(NT):
                        nc.tensor.matmul(ps[:], WkRes[:, dt_, t_out * P:(t_out + 1) * P],
                                         xb[:, dt_, :], start=(dt_ == 0), stop=(dt_ == NT - 1))
                    nc.vector.tensor_copy(kraw[:, t_out, :], ps[:])
                rope_block(KT[:, :, sl], kraw, cms[jb], sms[jb])
                nc.scalar.dma_start(kvinK[jb][:], KT[:, :, sl])
                nc.gpsimd.collective_compute(
                    "AllGather", mybir.AluOpType.bypass, replica_groups=RG,
                    ins=[kvinK[jb].opt()], outs=[kvoutK[jb].opt()])
                if jb == 1:
                    # WkRes dead after this chunk's K matmuls: reuse for Wq
                    WqRes = wres.tile([P, NT, D], BF16, tag="wk")
                    for hh in range(2):
                        w_sl = slice(hh * SB, (hh + 1) * SB)
                        nc.scalar.dma_start(WqRes[:, :, w_sl], inp["WqT"][hh])
                # V projection of my 512-key chunk
                for sk_ in range(SB // P):
                    for dh in range(2):
                        ps = psB.tile([P, SB], F32, tag="psB")
                        for dt_ in range(NT):
                            nc.tensor.matmul(ps[:], xb[:, dt_, sk_ * P:(sk_ + 1) * P],
                                             WvRes[:, dt_, dh * SB:(dh + 1) * SB],
                                             start=(dt_ == 0), stop=(dt_ == NT - 1))
                        nc.scalar.copy(V[:, jb * (SB // P) + sk_, dh * SB:(dh + 1) * SB], ps[:])
                nc.scalar.dma_start(kvinV[jb][:],
                                    V[:, jb * (SB // P):(jb + 1) * (SB // P), :])
                nc.gpsimd.collective_compute(
                    "AllGather", mybir.AluOpType.bypass, replica_groups=RG,
                    ins=[kvinV[jb].opt()], outs=[kvoutV[jb].opt()])
                if jb == 1:
                    # WvRes dead: reuse for Wo
                    WoRes = wres.tile([P, NT, D], BF16, tag="wv")
                    for hh in range(2):
                        w_sl = slice(hh * SB, (hh + 1) * SB)
                        nc.scalar.dma_start(WoRes[:, :, w_sl], inp["WoT"][hh])

            # ---------- gather AG results back to SBUF (gpsimd queues, so
            # the AG-gated waits never block the sync/scalar rings) ---------
            # ordered by AG completion (K0, V0, K1, V1) to avoid FIFO
            # head-of-line inversion on the gpsimd queue
            for jb in range(2):
                for r in range(2):
                    g = 2 * jb + r
                    nc.gpsimd.dma_start(KT[:, :, g * SB:(g + 1) * SB],
                                        kvoutK[jb][r])
                for r in range(2):
                    g = 2 * jb + r
                    nc.gpsimd.dma_start(V[:, g * (SB // P):(g + 1) * (SB // P), :],
                                        kvoutV[jb][r])

            # ---------- Phase 1: Q^T projection + RoPE (overlaps the AGs) ---
            for qh, (xq, cq, sq) in enumerate(((xq0, cq0, sq0), (xq1, cq1, sq1))):
                sl = slice(qh * SB, (qh + 1) * SB)
                qraw = rawp.tile([P, NT, SB], BF16, tag="raw")
                for t_out in range(NT):
                    ps = psB.tile([P, SB], F32, tag="psB")
                    for dt_ in range(NT):
                        nc.tensor.matmul(ps[:], WqRes[:, dt_, t_out * P:(t_out + 1) * P],
                                         xq[:, dt_, :], start=(dt_ == 0), stop=(dt_ == NT - 1))
                    nc.vector.tensor_copy(qraw[:, t_out, :], ps[:])
                rope_block(QT[:, :, sl], qraw, cq, sq)

            # ---------- attention + output projection, software-pipelined ---
            sumrows = {}
            rcps = {}

            def bc_part1(s, sumacc):
                # rowsum of slot s across partitions -> [1, QB] (PE trick)
                sums_ps = sps.tile([1, SB], F32, tag="sums")
                nc.tensor.matmul(sums_ps[0:1, 0:QB], ones_col[:], sumacc[:],
                                 start=True, stop=True)
                sumrow = smp.tile([1, QB], F32R, tag="sumrow")
                nc.scalar.copy(sumrow[:], sums_ps[0:1, 0:QB])
                sumrows[s] = sumrow

            def bc_part2(s):
                # broadcast rowsum to all partitions, reciprocal -> rcps[s]
                bc_ps = psB.tile([P, SB], F32, tag="psB")
                nc.tensor.matmul(bc_ps[:, 0:QB], ones_row[:], sumrows[s][:],
                                 start=True, stop=True)
                bc_sums = bcp.tile([P, QB], F32R, tag="bcs")
                nc.scalar.copy(bc_sums[:], bc_ps[:, 0:QB])
                rcp = bcp.tile([P, QB], F32, tag="bc")
                nc.vector.reciprocal(rcp[:], bc_sums[:])
                rcps[s] = rcp

            def o_quarter(qh):
                # output projection of slot qh's queries; normalization
                # (1/rowsum) fused into the PSUM eviction
                sl = slice(qh * QB, (qh + 1) * QB)
                ob = obp.tile([P, NT, QB], BF16, tag="ob")
                for oc in range(NT):
                    ps = psB.tile([P, SB], F32, tag="psB")
                    for dt_ in range(NT):
                        nc.tensor.matmul(ps[:, 0:QB], WoRes[:, dt_, oc * P:(oc + 1) * P],
                                         attnT[:, dt_, sl], start=(dt_ == 0), stop=(dt_ == NT - 1))
                    nc.vector.tensor_mul(out=ob[:, oc, :], in0=ps[:, 0:QB], in1=rcps[qh][:])
                    nc.sync.dma_start(outT[:, oc, sl], ob[:, oc, :])

            # Attention is split into two passes matched to AllGather arrival
            # order: pass A uses only key/value blocks 0,1 (AG K0/V0) — slots
            # 0,1 complete there; slots 2,3 run their first-8 score visits.
            # Pass B finishes slots 2,3 (scores on blocks 2,3, then all PV).
            # attnT (aliasing QT) is only written once a slot's scores are
            # fully done, so QT columns are never clobbered early.
            sumaccs = {}
            PTs = {}

            def scores_part(s, v0, v1):
                # score visits [v0, v1) of slot s -> PTs[(s, v0)], sumaccs[s]
                c = NVIS[s]
                qsl = slice(s * QB, (s + 1) * QB)
                if v0 == 0:
                    sumaccs[s] = sap.tile([P, QB], F32R, tag="sa", name=f"sa{s}")
                sumacc = sumaccs[s]
                PT = ptp.tile([P, 8, QB], BF16, tag="pts", name=f"PT{s}_{v0}")
                PTs[(s, v0)] = PT
                for v in range(v0, v1):
                    ps = psB.tile([P, SB], F32, tag="psB")
                    for dt_ in range(NT):
                        nc.tensor.matmul(ps[:, 0:QB], KT[:, dt_, v * P:(v + 1) * P],
                                         QT[:, dt_, qsl], start=(dt_ == 0), stop=(dt_ == NT - 1))
                    nc.scalar.activation(PT[:, v - v0, :], ps[:, 0:QB],
                                         mybir.ActivationFunctionType.Exp, scale=SCALE)
                    if v >= c - 4:
                        nc.vector.tensor_mul(out=PT[:, v - v0, :], in0=PT[:, v - v0, :],
                                             in1=maskst[:, 4 * s + (v - (c - 4)), :])
                    if v == 0:
                        nc.vector.tensor_copy(sumacc[:], PT[:, v - v0, :])
                    else:
                        nc.vector.tensor_tensor(sumacc[:], sumacc[:], PT[:, v - v0, :],
                                                mybir.AluOpType.add)

            def PTof(s, v):
                return PTs[(s, 0)] if v < 8 else PTs[(s, 8)], v % 8

            def pv_part(s, v0, v1, mode, partial=None, qoff=0):
                # both PV d-halves over visits [v0, v1) of slot s.
                # mode: 'attn'    -> copy into attnT (slot fully done)
                #       'partial' -> copy into the scratch partial buffer
                #       'combine' -> attnT = partial + psum (slot now done)
                qsl = slice(s * QB, (s + 1) * QB)
                for half in range(2):
                    pv = [psPV.tile([P, SB], F32, tag="pv", name=f"pv{s}_{v0}_{half}_{j2}")
                          for j2 in range(4)]
                    for v in range(v0, v1):
                        PT, row = PTof(s, v)
                        for j2 in range(4):
                            nc.tensor.matmul(pv[j2][:, 0:QB],
                                             V[:, v, (4 * half + j2) * P:(4 * half + j2 + 1) * P],
                                             PT[:, row, :], start=(v == v0),
                                             stop=(v == v1 - 1))
                    for j2 in range(4):
                        dt_ = 4 * half + j2
                        if mode == 'attn':
                            if half == 0:
                                nc.vector.tensor_copy(attnT[:, dt_, qsl], pv[j2][:, 0:QB])
                            else:
                                nc.scalar.copy(attnT[:, dt_, qsl], pv[j2][:, 0:QB])
                        elif mode == 'partial':
                            if half == 0:
                                nc.vector.tensor_copy(partial[:, dt_, qoff:qoff + QB],
                                                      pv[j2][:, 0:QB])
                            else:
                                nc.scalar.copy(partial[:, dt_, qoff:qoff + QB],
                                               pv[j2][:, 0:QB])
                        else:
                            nc.vector.tensor_tensor(attnT[:, dt_, qsl],
                                                    partial[:, dt_, qoff:qoff + QB],
                                                    pv[j2][:, 0:QB],
                                                    mybir.AluOpType.add)

            # ---- pass A: key/value blocks 0,1 ------------------------------
            # All pass-A scores run back-to-back first (they only need the
            # K0 gather); the PVs follow as a block once the V0 gather lands,
            # so the PE never idles mid-pass (idle >3.4us re-throttles HAM).
            scores_part(0, 0, 4)
            bc_part1(0, sumaccs[0])
            scores_part(1, 0, 8)
            bc_part1(1, sumaccs[1])
            scores_part(2, 0, 8)
            scores_part(3, 0, 8)
            pv_part(0, 0, 4, 'attn')
            pv_part(1, 0, 8, 'attn')
            bc_part2(0)
            o_quarter(0)
            bc_part2(1)
            o_quarter(1)
            # slots 2,3: PV over the early visits lands in a scratch buffer
            # (attnT would clobber QT columns their late scores still read)
            partial = rawp.tile([P, NT, SB], BF16, tag="raw", name="pvpart")
            pv_part(2, 0, 8, 'partial', partial, 0)
            pv_part(3, 0, 8, 'partial', partial, QB)

            # ---- pass B: key/value blocks 2,3 ------------------------------
            scores_part(2, 8, NVIS[2])
            bc_part1(2, sumaccs[2])
            scores_part(3, 8, NVIS[3])
            bc_part1(3, sumaccs[3])
            pv_part(2, 8, NVIS[2], 'combine', partial, 0)
            bc_part2(2)
            o_quarter(2)
            pv_part(3, 8, NVIS[3], 'combine', partial, QB)
            bc_part2(3)
            o_quarter(3)

    nc.finalize()
    return nc


def _chunk_pack(a, w=SB):
    # a: [R*P, C] -> [C//w, P, R, w] so each (chunk, partition) row is one
    # contiguous DRAM run (R*w elements) for line-rate DMA descriptors
    R = a.shape[0] // P
    nch = a.shape[1] // w
    return np.ascontiguousarray(a.reshape(R, P, nch, w).transpose(2, 1, 0, 3))


def _host_inputs(x, Wq, Wk, Wv, Wo, token_positions):
    import ml_dtypes
    bf = ml_dtypes.bfloat16
    perm = np.concatenate([np.arange(0, D, 2), np.arange(1, D, 2)])
    WqTp = _chunk_pack(Wq[perm].T.astype(bf))
    WkTp = _chunk_pack(Wk[perm].T.astype(bf), w=SB // 2)
    WvT = _chunk_pack(Wv.T.astype(bf))
    WoT = _chunk_pack(Wo.T.astype(bf))
    inv_freq = (1.0 / (np.float32(THETA) **
                       (np.arange(0, D, 2, dtype=np.float32) / np.float32(D))))
    ones_col = np.ones((P, 1), np.float32)
    ones_row = np.ones((1, P), np.float32)

    in_maps, metas = [], []
    for b in range(B):
        xT = np.ascontiguousarray(x[b].T).astype(bf)           # [D, S]
        pos = token_positions[b].astype(np.float32)
        ang = (pos[None, :] * inv_freq[:, None]).astype(np.float32)  # [D/2, S]
        cosF = np.cos(ang)
        sinF = np.sin(ang)
        for h in range(2):
            blocks = BLOCKS[h]
            qcols = np.concatenate([np.arange(QB * bs, QB * (bs + 1))
                                    for bs in blocks])
            xTq = _chunk_pack(xT[:, qcols])
            cosQ = _chunk_pack(cosF[:, qcols].astype(bf))
            sinQ = _chunk_pack(sinF[:, qcols].astype(bf))
            # my key half: global seq blocks h and 2+h (512 keys each)
            mcols = np.concatenate([np.arange(SB * h, SB * (h + 1)),
                                    np.arange(1024 + SB * h, 1024 + SB * (h + 1))])
            xTm = _chunk_pack(xT[:, mcols])
            cosM = _chunk_pack(cosF[:, mcols].astype(bf))
            sinM = _chunk_pack(sinF[:, mcols].astype(bf))
            m = np.zeros((P, 16, QB), dtype=np.float32)
            for s, bs in enumerate(blocks):
                c = NVIS[s]
                q0 = QB * bs
                q_glob = q0 + np.arange(QB)
                for j in range(4):
                    v = c - 4 + j
                    k_glob = 128 * v + np.arange(P)
                    m[:, 4 * s + j, :] = (q_glob[None, :] >= k_glob[:, None])
            in_maps.append({
                "ones_col": ones_col, "ones_row": ones_row,
                "xTm": xTm, "xTq": xTq,
                "WqT": WqTp, "WkT": WkTp, "WvT": WvT, "WoT": WoT,
                "cosM": cosM, "sinM": sinM,
                "cosQ": cosQ, "sinQ": sinQ,
                "masks": m.astype(bf),
            })
            metas.append((b, qcols))
    return in_maps, metas


_NC_CACHE = {}


def kernel(x, Wq, Wk, Wv, Wo, token_positions):
    x = np.asarray(x); token_positions = np.asarray(token_positions)
    if "nc" not in _NC_CACHE:
        _NC_CACHE["nc"] = _build_program()
    nc = _NC_CACHE["nc"]
    in_maps, metas = _host_inputs(np.asarray(x), np.asarray(Wq), np.asarray(Wk),
                                  np.asarray(Wv), np.asarray(Wo), token_positions)
    res = run_bass_kernel_spmd(nc, in_maps, core_ids=list(range(8)))
    out = np.empty((B, S, D), dtype=np.float32)
    for (b, qcols), r in zip(metas, res.results):
        oT = np.asarray(r["outT"]).astype(np.float32)   # [P, NT, NQ]
        o = np.transpose(oT, (2, 1, 0)).reshape(NQ, D)
        out[b, qcols, :] = o
    return out
